# revision 15
# baseline (speedup 1.0000x reference)
"""Multi-head attention (B=8, N=1024, C=768, H=12) on 8 TRN2 NeuronCores.

Sharding: data-parallel over batch — core i computes batch element i fully.
Weights / bias tables are replicated. No collectives.

Key ideas (all matmuls bf16, f32 PSUM accumulation):
  * Key compaction: the key_padding_mask invalidates ~half the keys. The host
    gathers the valid key rows of x into a compacted, zero-padded key-side
    input (NVpad rows), so S^T / exp / PV / V-projection shrink by ~NVpad/N.
    Padded rows carry zero V and a zero mask column, so they contribute
    nothing to either the numerator or the softmax denominator.
  * Flipped attention S^T[j,i] (compacted keys on partitions):
    P = exp(S^T) * ebias, where ebias is the host-gathered exp() of the
    relative-position bias for each (compacted key, query) pair
    (exp(S+B) = exp(S)exp(B)); no row-max subtraction needed (|S| <= ~10).
  * O^T_unnorm[d,i] and the softmax denominator come from ONE matmul per
    (jt, i-chunk): lhsT = [V | mask] puts the masked softmax sum in PSUM
    row 64.
  * Prioritized DMA order + per-pair weight slices let the softmax (Scalar
    exp) pipeline start ~14us in instead of ~45us.
  * Next pair's QKV projection work is interleaved chunk-wise into the
    attention jt-loop as PE "filler", so the PE never idles waiting on the
    exp -> ebias-mult pipeline.
  * Attention is software-pipelined across heads (head h+1's S^T/exp phase
    overlaps head h's PV tail) so the Scalar exp stream never drains; the
    denominator rows are staged to base-partition-0 tiles and inverted in
    batches (custom-DVE reciprocal for the early groups, Scalar ln/exp for
    the projection-gating last group), then broadcast via a DRAM bounce.
  * Output projection consumes O^T directly as lhsT (c-major layout); b_proj
    rides a rank-1 PSUM contraction step so evacuation is a plain copy split
    across Scalar/Vector; output tokens store as bf16 (well within the 2e-2
    tolerance) to halve the store tail.
"""

import functools
import itertools
from collections import deque

import numpy as np
import ml_dtypes

DIM = 768
NUM_HEADS = 12
HD = 64
N_TOK = 1024
B = 8
SCALE = HD ** -0.5

_BUILD_CACHE = {}


def _build_nc(N=N_TOK, H=NUM_HEADS, NVT=None, nv_last=128, mmdt_name="bfloat16"):
    import concourse.bass as bass
    import concourse.mybir as mybir
    import concourse.tile as tile
    from concourse import bacc

    # Pin every activation to the one table set containing both exp and ln,
    # so the Scalar engine never thrashes ACT_TABLE_LOADs between the softmax
    # exp stream and the ln/exp reciprocal. Other sets are emptied (indices
    # must stay aligned with act_info.json, so no reordering/filtering).
    if not getattr(bacc, "_act_tables_pinned", False):
        _orig_gat = bacc.get_activation_tables

        def _pinned_gat(arch):
            tabs = _orig_gat(arch)
            want = None
            for name, funcs in tabs.items():
                fn = {f.name.lower() for f in funcs}
                if "exp" in fn and "ln" in fn and "copy" in fn:
                    want = name
                    break
            if want is None:
                return tabs
            return {
                name: (funcs if name == want else set())
                for name, funcs in tabs.items()
            }

        bacc.get_activation_tables = _pinned_gat
        bacc._act_tables_pinned = True

    f32 = mybir.dt.float32
    mmdt = getattr(mybir.dt, mmdt_name)
    Exp = mybir.ActivationFunctionType.Exp
    Ln = mybir.ActivationFunctionType.Ln
    mult = mybir.AluOpType.mult
    add = mybir.AluOpType.add

    C = H * HD                      # 768
    NT = N // 128                   # query tiles
    if NVT is None:
        NVT = N // 128
    NV = NVT * 128
    KO = C // 128                   # contraction slots (== head pairs HP)
    HP = H // 2
    ichunks = [(i0, min(512, N - i0)) for i0 in range(0, N, 512)]
    kchunks = [(k0, min(512, NV - k0)) for k0 in range(0, NV, 512)]
    fchunks = [(f0, min(512, C - f0)) for f0 in range(0, C, 512)]

    nc = bacc.Bacc(None)
    xT_d = nc.declare_dram_parameter("xT", [C, N], mmdt, isOutput=False)
    xkT_d = nc.declare_dram_parameter("xkT", [C, NV], mmdt, isOutput=False)
    wqk_d = nc.declare_dram_parameter("wqkp", [C, 2 * C], mmdt, isOutput=False)
    wv_d = nc.declare_dram_parameter("wv", [C, C], mmdt, isOutput=False)
    wp_d = nc.declare_dram_parameter("wp", [C, C], mmdt, isOutput=False)
    ebias_d = nc.declare_dram_parameter("ebias", [H, NV, N], mmdt, isOutput=False)
    mask_d = nc.declare_dram_parameter("maskc", [128, NVT], mmdt, isOutput=False)
    bpb_d = nc.declare_dram_parameter("bproj", [1, C], mmdt, isOutput=False)
    out_d = nc.declare_dram_parameter("out", [N, C], mmdt, isOutput=True)

    with tile.TileContext(nc) as tc:
        with (
            tc.tile_pool(name="singles", bufs=1) as singles,
            tc.tile_pool(name="dram", bufs=1, space="DRAM") as drampool,
        ):
            # ---- input loads, critical-path first.  The first QK chunk only
            # needs the first column-halves of x / x_k plus pair-0 weights, so
            # those transfers are split column-wise and front-loaded. ----
            maskc = singles.tile([128, NVT], mmdt)
            nc.sync.dma_start(maskc[:], mask_d[:])
            xT = singles.tile([128, KO, N], mmdt)
            xT_r = xT_d.rearrange("(ko p) n -> p ko n", p=128)
            nc.sync.dma_start(xT[:, :, 0:512], xT_r[:, :, 0:512])
            # per-pair packed q|k weight slices: wqk[:, ko, pair, 0:128]=W_q,
            # [.., 128:256]=W_k.  Pair 0 first, rest after ebias h0.
            wqk = singles.tile([128, KO, HP, 256], mmdt)
            wqk_r = wqk_d.rearrange("(ko p) m -> p ko m", p=128)
            nc.sync.dma_start(wqk[:, :, 0, :], wqk_r[:, :, 0:256])
            xkt = singles.tile([128, KO, NV], mmdt)
            xkT_r = xkT_d.rearrange("(ko p) n -> p ko n", p=128)
            nc.sync.dma_start(xkt[:, :, 0:512], xkT_r[:, :, 0:512])
            wv = singles.tile([128, KO, C], mmdt)
            wv_r = wv_d.rearrange("(ko p) m -> p ko m", p=128)
            nc.sync.dma_start(wv[:, :, 0:512], wv_r[:, :, 0:512])
            nc.sync.dma_start(xT[:, :, 512:], xT_r[:, :, 512:])
            if NV > 512:
                nc.sync.dma_start(xkt[:, :, 512:], xkT_r[:, :, 512:])

            def xkTs(ko):
                return xkt[:, ko]

            qt = singles.tile([128, HP, N], mmdt)
            kt = singles.tile([128, HP, NV], mmdt)
            vsb = singles.tile([128, NVT, H, HD + 1], mmdt)
            ou = singles.tile([128, HP, N], mmdt)      # unnormalized O^T (packed)
            rb = singles.tile([128, HP, N], mmdt)      # broadcast recips (packed)
            # per-group denominator tiles; rows start at partition 0 because
            # the custom-DVE reciprocal only works at base partition 0
            dens = [singles.tile([128, N], f32, name=f"den{g}") for g in range(3)]
            rdens = [singles.tile([128, N], f32, name=f"rden{g}") for g in range(3)]
            rdbs = [singles.tile([128, N], mmdt, name=f"rdb{g}") for g in range(3)]
            rscratch = drampool.tile([H, N], mmdt)

            with (
                tc.tile_pool(name="qkv_psum", bufs=2, space="PSUM") as qp,
                tc.tile_pool(name="eb_pool", bufs=3) as eb_pool,
                tc.tile_pool(name="st_psum", bufs=2, space="PSUM") as st_psum,
                tc.tile_pool(name="pv_psum", bufs=len(ichunks), space="PSUM") as pv_psum,
                tc.tile_pool(name="e_pool", bufs=3) as e_pool,
                tc.tile_pool(name="p_pool", bufs=4) as p_pool,
                tc.tile_pool(name="drow_pool", bufs=3) as drow_pool,
            ):
                if HP >= 5:
                    # (first head, group index, n heads) by trigger pair
                    NORM_GROUPS = {
                        2: (0, 0, 6),
                        HP - 2: (6, 1, 2 * HP - 8),
                        HP - 1: (2 * HP - 2, 2, 2),
                    }
                else:
                    NORM_GROUPS = {HP - 1: (0, 0, H)}
                DEN_ROW = {}
                for _g0, _gi, _ng in NORM_GROUPS.values():
                    for _h in range(_g0, _g0 + _ng):
                        DEN_ROW[_h] = (_gi, _h - _g0)

                def _normalize_group(g0, gi, ng):
                    if gi < 2:
                        # early groups have pair-scale slack before their rb
                        # rows are needed: compute 1/x off the Scalar engine
                        # (custom-DVE seed+NR, base-partition-0 tiles) and cast
                        # to bf16 on the idle GpSimd engine
                        nc.vector.reciprocal_approx_fast(
                            rdens[gi][0:ng, :], dens[gi][0:ng, :]
                        )
                        nc.gpsimd.tensor_copy(
                            rdbs[gi][0:ng, :], rdens[gi][0:ng, :]
                        )
                    else:
                        # the last group gates the output projection: use the
                        # lowest-latency path (Scalar is idle by then)
                        nc.scalar.activation(
                            rdens[gi][0:ng, :], dens[gi][0:ng, :], Ln
                        )
                        nc.scalar.activation(
                            rdbs[gi][0:ng, :], rdens[gi][0:ng, :], Exp,
                            scale=-1.0,
                        )
                    nc.sync.dma_start(rscratch[g0 : g0 + ng], rdbs[gi][0:ng, :])
                    # two DMAs broadcast the whole group (even-head rows to
                    # partitions 0-63, odd-head rows to 64-127) instead of one
                    # per head, cutting Sync-queue bursts
                    for par in (0, 1):
                        nc.sync.dma_start(
                            rb[64 * par : 64 * par + 64, g0 // 2 : (g0 + ng) // 2, :],
                            bass.AP(
                                tensor=rscratch.tensor,
                                offset=rscratch[g0 + par, 0].offset,
                                ap=[[0, 64], [2 * N, ng // 2], [1, N]],
                            ),
                        )
                    for sl in range(g0 // 2, (g0 + ng) // 2):
                        nc.vector.tensor_tensor(
                            ou[:, sl, :], ou[:, sl, :], rb[:, sl, :], mult
                        )

                from concourse.tile import add_dep_helper

                ebt = {}

                def alloc_eb(h):
                    # padded key rows beyond nv_last of the last jt tile are
                    # never read (the ebias multiply is partition-sliced), so
                    # they are neither streamed nor zeroed
                    ebt[h] = eb_pool.tile(
                        [128, NVT, N], mmdt, tag="eb", name=f"eb_{h}"
                    )
                    ebr = ebias_d[h].rearrange("(jt p) n -> p jt n", p=128)
                    if nv_last < 128:
                        d1 = nc.sync.dma_start(
                            ebt[h][:, : NVT - 1, :], ebr[:, : NVT - 1, :]
                        )
                        d2 = nc.sync.dma_start(
                            ebt[h][:nv_last, NVT - 1, :], ebr[:nv_last, NVT - 1, :]
                        )
                        return [d1, d2]
                    return [nc.sync.dma_start(ebt[h][:], ebr)]

                # ebias head 0 arrives per-jt so the first ebias multiply only
                # waits for its own key tile, not the whole head
                ebt[0] = eb_pool.tile([128, NVT, N], mmdt, tag="eb", name="eb_0")
                eb0_r = ebias_d[0].rearrange("(jt p) n -> p jt n", p=128)
                eb0_dmas = [
                    nc.sync.dma_start(
                        ebt[0][: (nv_last if jt == NVT - 1 else 128), jt, :],
                        eb0_r[: (nv_last if jt == NVT - 1 else 128), jt, :],
                    )
                    for jt in range(NVT)
                ]
                wave3_dmas = list(alloc_eb(1))
                wave3_dmas.append(
                    nc.sync.dma_start(wqk[:, :, 1:, :], wqk_r[:, :, 256:])
                )
                wave3_dmas.append(
                    nc.sync.dma_start(wv[:, :, 512:], wv_r[:, :, 512:])
                )
                wp = singles.tile([128, KO, C], mmdt)
                wave3_dmas.append(
                    nc.sync.dma_start(
                        wp[:], wp_d.rearrange("(ko p) m -> p ko m", p=128)
                    )
                )
                # bias as a bf16 row + a ones row: the output projection adds
                # b_proj via one extra rank-1 contraction step in PSUM, so the
                # PSUM evacuation becomes a plain copy splittable across the
                # Scalar and Vector engines
                bpb = singles.tile([1, C], mmdt)
                wave3_dmas.append(nc.sync.dma_start(bpb[:], bpb_d[:]))
                ones = singles.tile([1, 128], mmdt)
                nc.vector.memset(ones[:], 1.0)

                # PE warm-up: dummy matmuls on constant data while the input
                # DMAs stream, so the HAM clock gate is open (2.4 GHz) when
                # the first real matmul issues.
                warm = e_pool.tile([128, 512], mmdt, tag="e", name="warm_in")
                nc.vector.memset(warm[:], 0.0)
                wps = qp.tile([128, 512], f32, tag="ps", name="warm_ps")
                for _w in range(12):
                    nc.tensor.matmul(
                        wps[:],
                        lhsT=warm[:, :128],
                        rhs=warm[:],
                        start=True,
                        stop=True,
                    )

                # masked "ones" column (accumulates the softmax denominator);
                # compacted V rows for padded keys are already zero (zero x),
                # so no V masking is needed.
                nc.vector.tensor_scalar_mul(
                    vsb[:, :, :, HD : HD + 1],
                    maskc[:, :, None, None].to_broadcast([128, NVT, H, 1]),
                    1.0,
                )

                # ---- projection work units (also used as PE filler).  Each
                # QK chunk splits into two half-contraction pops so a filler
                # pop never occupies the PE long enough to starve the exp
                # pipeline. ----
                KOH = KO // 2
                qk0_mms = []   # pair-0 first-ko matmuls, used as DMA anchors

                def qk_half(pr, side, c0, cl, ko0, ko1, state):
                    if ko0 == 0:
                        state["ps"] = qp.tile(
                            [128, 512], f32, tag="ps",
                            name=f"qk_{pr}_{side}_{c0}",
                        )
                    ps = state["ps"]
                    for ko in range(ko0, ko1):
                        rhs = (
                            xT[:, ko, c0 : c0 + cl]
                            if side == 0
                            else xkTs(ko)[:, c0 : c0 + cl]
                        )
                        mm = nc.tensor.matmul(
                            ps[:, :cl],
                            lhsT=wqk[:, ko, pr, 128 * side : 128 * side + 128],
                            rhs=rhs,
                            start=(ko == 0),
                            stop=(ko == KO - 1),
                        )
                        if pr == 0 and ko == 0:
                            qk0_mms.append(mm)
                    if ko1 == KO:
                        dst = qt if side == 0 else kt
                        nc.vector.tensor_copy(
                            dst[:, pr, c0 : c0 + cl], ps[:, :cl]
                        )

                def qk_units(pr, side, c0, cl):
                    # wide chunks split into two half-contraction pops so a
                    # filler never occupies the PE long enough to starve exp
                    state = {}
                    if cl > 256:
                        return [
                            functools.partial(qk_half, pr, side, c0, cl, 0, KOH, state),
                            functools.partial(qk_half, pr, side, c0, cl, KOH, KO, state),
                        ]
                    return [functools.partial(qk_half, pr, side, c0, cl, 0, KO, state)]

                # V projection: out [128 keys, fl vdims] per (jt, fchunk);
                # wide rhs -> 6 matmuls + 1 evac per unit (vs 12+1 at 128)
                v_evac_alt = itertools.cycle([0, 1])

                def v_unit(jt, f0, fl):
                    ps = qp.tile(
                        [128, 512], f32, tag="ps", name=f"v_{jt}_{f0}"
                    )
                    for ko in range(KO):
                        nc.tensor.matmul(
                            ps[:, :fl],
                            lhsT=xkTs(ko)[:, 128 * jt : 128 * jt + 128],
                            rhs=wv[:, ko, f0 : f0 + fl],
                            start=(ko == 0),
                            stop=(ko == KO - 1),
                        )
                    evac = (
                        nc.scalar.copy if next(v_evac_alt) else nc.vector.tensor_copy
                    )
                    evac(
                        vsb[:, jt, f0 // HD : (f0 + fl) // HD, 0:HD],
                        ps[:, :fl].rearrange("p (h d) -> p h d", d=HD),
                    )

                def pair_units(pr):
                    units = []
                    for i0, il in ichunks:
                        units.extend(qk_units(pr, 0, i0, il))
                    for k0, kl in kchunks:
                        units.extend(qk_units(pr, 1, k0, kl))
                    return deque(units)

                # pair 0's projections + first V key tiles run up front
                # (nothing to hide behind); remaining V rides as filler
                for u in pair_units(0):
                    u()
                v_unit(0, 0, 512)
                v_unit(1, 0, 512)
                vq512 = [
                    functools.partial(v_unit, jt, 0, 512)
                    for jt in range(2, NVT)
                ]
                vq256 = [
                    functools.partial(v_unit, jt, 512, C - 512)
                    for jt in range(NVT)
                ]
                # late bulk DMAs start only once the critical-set transfers
                # have landed: ebias h0 rides behind the 3rd pair-0 chunk, the
                # big weight bulk behind the first S^T (attached post-driver),
                # so neither steals bandwidth from the startup critical path.
                for d in eb0_dmas:
                    add_dep_helper(
                        d.ins, qk0_mms[0].ins, sync=True,
                        reason="ebias h0 behind critical startup loads",
                    )
                anchor_mms = []

                # ---- software-pipelined attention: head h+1's S^T/exp phase
                # overlaps head h's PV tail, so the Scalar exp stream never
                # drains at head boundaries ----
                def make_head(h):
                    hp, ho = h // 2, 64 * (h % 2)
                    ctx = {"ptiles": {}, "pvs": None}

                    def s_stage(jt):
                        st = st_psum.tile([128, N], f32, tag="st")
                        for i0, il in ichunks:
                            mm = nc.tensor.matmul(
                                st[:, i0 : i0 + il],
                                lhsT=kt[ho : ho + 64, hp, 128 * jt : 128 * jt + 128],
                                rhs=qt[ho : ho + 64, hp, i0 : i0 + il],
                                start=True,
                                stop=True,
                            )
                            if h == 0 and jt == 0 and not anchor_mms:
                                anchor_mms.append(mm)
                        e = e_pool.tile([128, N], mmdt, tag="e")
                        nc.scalar.activation(e[:], st[:], Exp)
                        p = p_pool.tile([128, N], mmdt, tag="p")
                        # only the valid key rows are multiplied; padded rows
                        # of the last jt tile keep stale-but-finite values
                        # (pool regions are pre-touched once), and multiply
                        # zero V/mask columns in the PV matmul
                        if jt == NVT - 1:
                            if h < 4:
                                # 32-row chunks (alignment rule); valid rows
                                # are rewritten by the multiply below
                                for p0 in range((nv_last // 32) * 32, 128, 32):
                                    nc.vector.memset(p[p0 : p0 + 32, :], 0.0)
                            rows = nv_last
                        else:
                            rows = 128
                        mul_eng = (
                            nc.gpsimd
                            if (jt == 2 or (h % 2 == 1 and jt == 0))
                            else nc.vector
                        )
                        mul_eng.tensor_tensor(
                            p[:rows, :], e[:rows, :], ebt[h][:rows, jt, :], mult
                        )
                        ctx["ptiles"][jt] = p

                    def p_stage(jd):
                        if ctx["pvs"] is None:
                            ctx["pvs"] = [
                                pv_psum.tile(
                                    [128, 512], f32, tag="pv", name=f"pv_{h}_{ic}"
                                )
                                for ic in range(len(ichunks))
                            ]
                        pd = ctx["ptiles"].pop(jd)
                        for ic, (i0, il) in enumerate(ichunks):
                            nc.tensor.matmul(
                                ctx["pvs"][ic][: HD + 1, :il],
                                lhsT=vsb[:, jd, h, :],
                                rhs=pd[:, i0 : i0 + il],
                                start=(jd == 0),
                                stop=(jd == NVT - 1),
                            )

                    def evac():
                        pvs = ctx["pvs"]
                        for ic, (i0, il) in enumerate(ichunks):
                            nc.vector.tensor_copy(
                                ou[ho : ho + 64, hp, i0 : i0 + il],
                                pvs[ic][:HD, :il],
                            )
                            drow = drow_pool.tile(
                                [128, 512], f32, tag="drow", name=f"drow_{h}_{ic}"
                            )
                            # 1-of-4 denominator evacs per pair on Scalar, the
                            # rest on Vector (engine writes must land on an
                            # aligned partition base, hence the 64-row bounce)
                            if h % 2 == 0 and ic == 0:
                                nc.scalar.copy(
                                    drow[64:65, :il], pvs[ic][HD : HD + 1, :il]
                                )
                            else:
                                nc.vector.tensor_copy(
                                    drow[64:65, :il], pvs[ic][HD : HD + 1, :il]
                                )
                            gi, dr = DEN_ROW[h]
                            nc.gpsimd.dma_start(
                                dens[gi][dr : dr + 1, i0 : i0 + il],
                                drow[64:65, :il],
                            )
                        if h % 2 == 1 and h // 2 in NORM_GROUPS:
                            _normalize_group(*NORM_GROUPS[h // 2])

                    return s_stage, p_stage, evac

                heads = [make_head(h) for h in range(H)]
                filler = deque()
                _vextra = {
                    0: vq512,
                    2: vq256[:3],
                    4: vq256[3:],
                }
                for h in range(H):
                    s_stage, p_stage, _ = heads[h]
                    if h % 2 == 0:
                        filler = (
                            pair_units(h // 2 + 1)
                            if h // 2 + 1 < HP
                            else deque()
                        )
                        filler.extendleft(reversed(_vextra.get(h, [])))
                    for k in range(NVT):
                        s_stage(k)
                        if k < 2 and h > 0:
                            heads[h - 1][1](NVT - 2 + k)
                            if k == 1:
                                heads[h - 1][2]()
                        if k >= 2:
                            p_stage(k - 2)
                        if k == 2 and h + 2 < H:
                            alloc_eb(h + 2)
                        if filler:
                            filler.popleft()()
                # drain the last head's PV tail
                heads[H - 1][1](NVT - 2)
                heads[H - 1][1](NVT - 1)
                heads[H - 1][2]()
                for d in wave3_dmas:
                    add_dep_helper(
                        d.ins, qk0_mms[-1].ins, sync=True,
                        reason="bulk loads behind startup + ebias h0",
                    )

            # ---------------- output projection ----------------
            # ko 0..KO-2 run eagerly (their O^T slots normalize early); each
            # group's final-ko matmul (the slot normalized in the very last
            # group) is lagged so the normalize tail hides under real work.
            with (
                tc.tile_pool(name="proj_psum", bufs=7, space="PSUM") as proj_psum,
                tc.tile_pool(name="o_pool", bufs=3) as o_pool,
            ):
                groups = [(it, f0, fl) for it in range(NT) for f0, fl in fchunks]
                LAG = min(6, len(groups) - 1) if KO > 1 else 0
                psums = {}
                ots = {}
                for g in range(len(groups) + LAG):
                    if g < len(groups):
                        it, f0, fl = groups[g]
                        ps = proj_psum.tile(
                            [128, 512], f32, tag="ps", name=f"pps_{g}"
                        )
                        for ko in range(KO - 1):
                            nc.tensor.matmul(
                                ps[:, :fl],
                                lhsT=ou[:, ko, 128 * it : 128 * it + 128],
                                rhs=wp[:, ko, f0 : f0 + fl],
                                start=(ko == 0),
                                stop=False,
                            )
                        # rank-1 step adds b_proj into the accumulation
                        nc.tensor.matmul(
                            ps[:, :fl],
                            lhsT=ones[:],
                            rhs=bpb[:, f0 : f0 + fl],
                            start=False,
                            stop=False,
                        )
                        psums[g] = ps
                    if g >= LAG:
                        it, f0, fl = groups[g - LAG]
                        ps = psums.pop(g - LAG)
                        nc.tensor.matmul(
                            ps[:, :fl],
                            lhsT=ou[:, KO - 1, 128 * it : 128 * it + 128],
                            rhs=wp[:, KO - 1, f0 : f0 + fl],
                            start=(KO == 1),
                            stop=True,
                        )
                        if it not in ots:
                            ots[it] = o_pool.tile(
                                [128, C], mmdt, tag="ot", name=f"ot_{it}"
                            )
                        # bias already in PSUM: evacuation is a plain copy,
                        # alternated across the two idle-in-tail engines
                        evac = (
                            nc.scalar.copy
                            if (g - LAG) % 2 == 0
                            else nc.vector.tensor_copy
                        )
                        evac(ots[it][:, f0 : f0 + fl], ps[:, :fl])
                        if f0 + fl >= C:
                            nc.sync.dma_start(
                                out_d[128 * it : 128 * it + 128, :], ots.pop(it)[:]
                            )

    nc.finalize()
    return nc


def _host_pack(x, w_qkv, w_proj, b_proj, bias_table, key_padding_mask,
               N=N_TOK, H=NUM_HEADS, mmdt_name="bfloat16"):
    """Host-side layout: per-core input dicts (core i <- batch i).
    Returns (in_maps, NVT)."""
    np_mmdt = ml_dtypes.bfloat16 if mmdt_name == "bfloat16" else np.float32
    C = H * HD
    HP = H // 2

    x = np.asarray(x, np.float32)
    mask = np.asarray(key_padding_mask).astype(bool)
    Bb = x.shape[0]

    valid = [np.where(mask[b])[0] for b in range(Bb)]
    nv_max = max(1, max(len(v) for v in valid))
    NVT = (nv_max + 127) // 128
    NV = NVT * 128

    w_qkv = np.asarray(w_qkv, np.float32)
    wqk = np.ascontiguousarray(w_qkv[: 2 * C].T).astype(np.float32)
    wqk[:, :C] *= SCALE                       # fold softmax scale into W_q
    # per-pair packed [C, HP, 256]: q slice | k slice for each head pair
    wqkp = np.empty((C, HP, 256), np.float32)
    for p in range(HP):
        wqkp[:, p, :128] = wqk[:, 128 * p : 128 * p + 128]
        wqkp[:, p, 128:] = wqk[:, C + 128 * p : C + 128 * p + 128]
    wqkp = np.ascontiguousarray(wqkp.reshape(C, 2 * C)).astype(np_mmdt)
    wv = np.ascontiguousarray(w_qkv[2 * C :].T).astype(np_mmdt)
    wp = np.ascontiguousarray(np.asarray(w_proj, np.float32).T).astype(np_mmdt)
    bpb = np.asarray(b_proj, np.float32).reshape(1, -1).astype(np_mmdt)

    etab = np.exp(np.asarray(bias_table, np.float32))   # [2N-1, H]
    iota = np.arange(N)

    in_maps = []
    for b in range(Bb):
        v = valid[b]
        nv = len(v)
        xT = np.ascontiguousarray(x[b].T).astype(np_mmdt)
        xk = np.zeros((NV, C), np.float32)
        xk[:nv] = x[b][v]
        xkT = np.ascontiguousarray(xk.T).astype(np_mmdt)
        mc = np.zeros(NV, np.float32)
        mc[:nv] = 1.0
        maskc = np.ascontiguousarray(mc.reshape(NVT, 128).T).astype(np_mmdt)
        # ebias[h, r, i] = exp(bias_table[v[r] - i + N - 1, h])
        idx = np.zeros((NV, N), np.int32)
        idx[:nv] = v[:, None] - iota[None, :] + N - 1
        eb = etab[idx, :]                     # [NV, N, H]
        eb[nv:] = 0.0
        ebias = np.ascontiguousarray(eb.transpose(2, 0, 1)).astype(np_mmdt)
        in_maps.append({
            "xT": xT, "xkT": xkT, "wqkp": wqkp, "wv": wv, "wp": wp,
            "ebias": ebias, "maskc": maskc, "bproj": bpb,
        })
    return in_maps, NVT


def _run(x, w_qkv, w_proj, b_proj, bias_table, key_padding_mask, trace=False):
    from concourse.bass_utils import run_bass_kernel_spmd

    in_maps, NVT = _host_pack(
        x, w_qkv, w_proj, b_proj, bias_table, key_padding_mask
    )
    nv_max = max(1, max(int(np.asarray(key_padding_mask)[b].sum())
                        for b in range(np.asarray(key_padding_mask).shape[0])))
    nv_last = nv_max - 128 * (NVT - 1)
    key = ("full", N_TOK, NUM_HEADS, NVT, nv_last)
    if key not in _BUILD_CACHE:
        _BUILD_CACHE[key] = _build_nc(NVT=NVT, nv_last=nv_last)
    nc = _BUILD_CACHE[key]
    res = run_bass_kernel_spmd(nc, in_maps, core_ids=list(range(B)), trace=trace)
    out = np.stack([np.asarray(res.results[i]["out"]) for i in range(B)])
    return out.astype(np.float32), res


def kernel(x, w_qkv, w_proj, b_proj, bias_table, key_padding_mask):
    out, _ = _run(x, w_qkv, w_proj, b_proj, bias_table, key_padding_mask)
    return out

